# revision 30
# baseline (speedup 1.0000x reference)
"""GCN+JumpingKnowledge distributed Trainium2 kernel (8 NeuronCores).

Strategy: shard destination nodes across 8 cores (6250 each). Per layer:
  - sharded feature transform: z rows = act^T chunks @ W on TensorE,
    stored row-major to HBM shard, AllGather -> z_full [50000,128] fp16
  - dma_gather source rows for this core's edges (sorted by dst tile,
    split by src < 32768 for int16 gather indices, padded to a common
    per-(tile,half) block count across cores so one SPMD program fits
    all). Gathers round-robin across 4 SWDGE queues so descriptor
    generation parallelizes over the 4 Q7 core pairs.
  - segment-sum via TensorE: psum[feat,dst] += G_blk^T @ S_blk where
    S_blk[e,d] = one_hot(dstoff_e) * normval_e is HOST-precomputed
    (graph-static, shared across layers) and streamed from HBM on the
    sync-engine DMA queues, freeing DVE entirely.
  - BN stats via per-tile accumulators + 1KB AllReduce, fused
    scale/shift/ReLU on ACT; JK max fused; final projection on device.
"""

import os
import sys

import numpy as np

sys.path.insert(0, "/opt/trn_rl_repo")

N = 50000
E = 800000
F = 128
OUTF = 64
N_CORES = 8
SHARD = N // N_CORES  # 6250
TILE = 128
NTILE = (SHARD + TILE - 1) // TILE  # 49
LAST_W = SHARD - (NTILE - 1) * TILE  # 106
PIECE = SHARD // 2  # 3125: rows-per-shard half for the 2-piece AllGather
NPC = N // 2  # 25000 rows per gathered piece (int16-indexable)
GRP = 4  # tiles per gather group
BN_EPS = 1e-5
ZCHUNK = 512
NQ = 4  # SWDGE queues (4 Q7 core pairs)


def _preprocess(edge_index):
    """Host-side edge routing. Returns (structure, per_core_arrays).

    Self-loops are NOT routed through the gather: each core applies them
    locally via a diagonal S block against its own z rows. Real edges are
    laid out per (dst tile, src half) with padding at the END of each
    (tile, half) segment; gather calls are aligned to those segments so
    pad slots (idx = -1, trailing in their call) are dropped by the
    gather ucode (fewer descriptors). S is zero there, and the gather
    buffers are memset once at startup, so dropped slots contribute 0.
    """
    src = np.asarray(edge_index[0], dtype=np.int64)
    dst = np.asarray(edge_index[1], dtype=np.int64)

    deg = np.bincount(dst, minlength=N).astype(np.float64) + 1.0
    dinv = 1.0 / np.sqrt(deg)

    normval = (dinv[src] * dinv[dst]).astype(np.float32)
    selfnorm = (dinv * dinv).astype(np.float32)  # per-node self-loop weight

    core = dst // SHARD
    tile_id = (dst % SHARD) // TILE
    # src piece: rows [p*PIECE, (p+1)*PIECE) of every shard are allgathered
    # into z_piece_p [NPC, F]; local row = shard*PIECE + (r % PIECE) < 32768.
    src_r = src % SHARD
    half = src_r // PIECE  # piece id (0/1), keeps the lo/hi naming downstream
    src_local = (src // SHARD) * PIECE + (src_r % PIECE)
    dstoff = ((dst % SHARD) % TILE).astype(np.int64)

    # per (core, tile, half) counts
    key = (core * NTILE + tile_id) * 2 + half
    counts = np.bincount(key, minlength=N_CORES * NTILE * 2).reshape(
        N_CORES, NTILE, 2
    )
    maxcnt = counts.max(axis=0)  # [NTILE, 2]
    pad_blocks = (maxcnt + TILE - 1) // TILE  # blocks per (tile, half)

    # slot layout: groups of GRP tiles; per group all lo segments then all
    # hi. gather calls are aligned to (tile, half) segments and chunked to
    # <= MAX_CALL slots (hw limit 1024 idxs/call) so per-core trailing pad
    # (idx=-1) is dropped by the ucode.
    MAX_CALL = 1024
    groups = []
    slot_start = np.zeros((NTILE, 2), dtype=np.int64)
    cursor = 0
    for g0 in range(0, NTILE, GRP):
        tiles = list(range(g0, min(g0 + GRP, NTILE)))
        ginfo = {"tiles": tiles}
        for h, nm in ((0, "lo"), (1, "hi")):
            run_slot0 = cursor
            tb = []
            calls = []
            for t in tiles:
                slot_start[t, h] = cursor
                tb.append((cursor, int(pad_blocks[t, h])))
                seg = int(pad_blocks[t, h]) * TILE
                o = cursor
                while o < cursor + seg:
                    n = min(MAX_CALL, cursor + seg - o)
                    calls.append((o, n))
                    o += n
                cursor += seg
            ginfo[nm] = {
                "slot0": run_slot0,
                "nslots": cursor - run_slot0,
                "tile_blocks": tb,
                "calls": calls,
            }
        groups.append(ginfo)
    total_slots = cursor
    total_blocks = total_slots // TILE

    # per-core slot content
    per_core = []
    for c in range(N_CORES):
        m = core == c
        e_t = tile_id[m]
        e_h = half[m]
        e_src = src_local[m]
        e_nv = normval[m]
        e_do = dstoff[m]
        order = np.lexsort((e_h, e_t))
        e_t, e_h = e_t[order], e_h[order]
        e_src, e_nv, e_do = e_src[order], e_nv[order], e_do[order]
        # rank within (t, h) group
        k = e_t * 2 + e_h
        cnt_c = np.bincount(k, minlength=NTILE * 2)
        grp_starts = np.concatenate([[0], np.cumsum(cnt_c)[:-1]])
        rank = np.arange(len(k)) - grp_starts[k]
        slots = slot_start[e_t, e_h] + rank

        # pad slots gather row 0 (valid, harmless: S is 0 there). idx=-1
        # trailing-drop wedges the device — do not use.
        padv = int(os.environ.get("KGNN_PADV", "0"))
        idx_vals = np.full(total_slots, padv, dtype=np.int16)
        idx_vals[slots] = e_src.astype(np.int16)

        # host-built S: S[slot%128, (slot//128)*128 + dstoff] = normval
        s_host = np.zeros((128, total_slots), dtype=np.float16)
        s_host[slots % TILE, (slots // TILE) * TILE + e_do] = e_nv

        # diagonal self-loop S: sdiag[p, t*128+p] = selfnorm of local node
        sdiag = np.zeros((128, NTILE * TILE), dtype=np.float16)
        loc = np.arange(SHARD)
        sdiag[loc % TILE, (loc // TILE) * TILE + loc % TILE] = selfnorm[
            c * SHARD : (c + 1) * SHARD
        ]

        # idx wrapped layout: slot i -> partition i%16 (replicated x8), col i//16
        idx_arr = np.zeros((128, total_slots // 16), dtype=np.int16)
        v16 = idx_vals.reshape(-1, 16).T  # [16, total/16]
        for g in range(8):
            idx_arr[16 * g : 16 * g + 16] = v16
        per_core.append({"idx": idx_arr, "S": s_host, "Sdiag": sdiag})

    structure = {
        "groups": groups,
        "total_slots": total_slots,
        "total_blocks": total_blocks,
    }
    return structure, per_core


def _build(structure):
    import concourse.bacc as bacc
    import concourse.tile as tile
    from concourse import mybir
    import concourse.bass as bass

    fp32 = mybir.dt.float32
    fp16 = mybir.dt.float16
    i16 = mybir.dt.int16
    AF = mybir.ActivationFunctionType
    OP = mybir.AluOpType

    groups = structure["groups"]
    total_slots = structure["total_slots"]

    nc = bacc.Bacc(
        "TRN2",
        target_bir_lowering=False,
        num_devices=N_CORES,
        num_swdge_queues=NQ,
    )

    # ---- I/O ----
    xT_in = nc.declare_dram_parameter("xT", [F, SHARD], fp16, isOutput=False)
    idx_in = nc.declare_dram_parameter("idx", [128, total_slots // 16], i16, isOutput=False)
    s_in = nc.declare_dram_parameter("S", [128, total_slots], fp16, isOutput=False)
    sdiag_in = nc.declare_dram_parameter("Sdiag", [128, NTILE * TILE], fp16, isOutput=False)
    w_in = [
        nc.declare_dram_parameter(f"W{i}", [F, F], fp16, isOutput=False)
        for i in (1, 2, 3)
    ]
    wp_in = nc.declare_dram_parameter("Wp", [F, OUTF], fp16, isOutput=False)
    b_in = [
        nc.declare_dram_parameter(f"b{i}", [F, 1], fp32, isOutput=False)
        for i in (1, 2, 3)
    ]
    bp_in = nc.declare_dram_parameter("bp", [OUTF, 1], fp32, isOutput=False)
    g_in = [
        nc.declare_dram_parameter(f"g{i}", [F, 1], fp32, isOutput=False) for i in (1, 2)
    ]
    be_in = [
        nc.declare_dram_parameter(f"be{i}", [F, 1], fp32, isOutput=False)
        for i in (1, 2)
    ]
    out_ext = nc.declare_dram_parameter("outT", [OUTF, SHARD], fp32, isOutput=True)

    with tile.TileContext(nc) as tc:
        from contextlib import ExitStack

        with ExitStack() as ctx:
            dram = ctx.enter_context(tc.tile_pool(name="dram", bufs=1, space="DRAM"))
            singles = ctx.enter_context(tc.tile_pool(name="singles", bufs=1))
            glo_p = ctx.enter_context(tc.tile_pool(name="glo", bufs=8))
            ghi_p = ctx.enter_context(tc.tile_pool(name="ghi", bufs=8))
            slo_p = ctx.enter_context(tc.tile_pool(name="slo", bufs=8))
            shi_p = ctx.enter_context(tc.tile_pool(name="shi", bufs=8))
            conv_ps = ctx.enter_context(tc.tile_pool(name="convps", bufs=3, space="PSUM"))
            z_ps = ctx.enter_context(tc.tile_pool(name="zps", bufs=4, space="PSUM"))
            zstage = ctx.enter_context(tc.tile_pool(name="zstage", bufs=4))
            rstage = ctx.enter_context(tc.tile_pool(name="rstage", bufs=3))
            small = ctx.enter_context(tc.tile_pool(name="small", bufs=2))

            # DRAM internals
            z_shards = [dram.tile([SHARD, F], fp16, name=f"z_shard{i}") for i in range(3)]
            z_pieces = [
                [
                    dram.tile([NPC, F], fp16, addr_space="Shared", name=f"z_piece{i}_{p}")
                    for p in range(2)
                ]
                for i in range(3)
            ]
            stats_locs = [dram.tile([F, 2], fp32, name=f"stats_loc{i}") for i in range(2)]
            stats_globs = [dram.tile([F, 2], fp32, addr_space="Shared", name=f"stats_glob{i}") for i in range(2)]

            # ---- load constants ----
            idx_sb = singles.tile([128, total_slots // 16], i16)
            nc.sync.dma_start(out=idx_sb[:], in_=idx_in[:])
            sdiag_sb = singles.tile([128, NTILE * TILE], fp16)
            nc.sync.dma_start(out=sdiag_sb[:], in_=sdiag_in[:])
            w_sb = []
            for i in range(3):
                w = singles.tile([F, F], fp16, name=f"w{i}")
                nc.sync.dma_start(out=w[:], in_=w_in[i][:])
                w_sb.append(w)
            wp_sb = singles.tile([F, OUTF], fp16)
            nc.sync.dma_start(out=wp_sb[:], in_=wp_in[:])
            b_sb = []
            for i in range(3):
                b = singles.tile([F, 1], fp32, name=f"b{i}")
                nc.sync.dma_start(out=b[:], in_=b_in[i][:])
                b_sb.append(b)
            bp_sb = singles.tile([OUTF, 1], fp32)
            nc.sync.dma_start(out=bp_sb[:], in_=bp_in[:])
            g_sb, be_sb = [], []
            for i in range(2):
                g = singles.tile([F, 1], fp32, name=f"g{i}")
                nc.sync.dma_start(out=g[:], in_=g_in[i][:])
                g_sb.append(g)
                be = singles.tile([F, 1], fp32, name=f"be{i}")
                nc.sync.dma_start(out=be[:], in_=be_in[i][:])
                be_sb.append(be)

            # persistent activations
            actA = singles.tile([F, SHARD], fp16)  # layer input act^T
            nc.sync.dma_start(out=actA[:], in_=xT_in[:])
            actB = singles.tile([F, SHARD], fp16)
            conv_sb = singles.tile([F, SHARD], fp32)
            stats6 = singles.tile([F, NTILE, 6], fp32)
            # this core's z rows, per tile: [node-in-tile, tile, feature]
            z_own = singles.tile([128, NTILE, F], fp16)
            nc.vector.memset(z_own[:], 0.0)

            # warm the gather pools: trailing-dropped pad slots leave SBUF
            # unwritten; ensure it is 0.0 (not NaN garbage) at first use.
            MAXBLK = 8
            for pool, tg in ((glo_p, "glo"), (ghi_p, "ghi")):
                for _ in range(8):
                    wt = pool.tile([128, MAXBLK, F], fp16, tag=tg)
                    nc.vector.memset(wt[:], 0.0)

            def produce_z(act_src, w_idx):
                """z rows = (act^T chunk)^T @ W -> z_own + z_shard; 2-piece
                allgather so piece-0 gathers can start while piece 1 flies."""
                z_shard = z_shards[w_idx]
                piece_end_tile = [(PIECE + TILE - 1) // TILE, NTILE]  # 25, 49
                t0 = 0
                for p in range(2):
                    for t in range(t0, piece_end_tile[p]):
                        o = t * TILE
                        w = LAST_W if t == NTILE - 1 else TILE
                        zp = z_ps.tile([TILE, F], fp32, tag="zps")
                        nc.tensor.matmul(
                            zp[:w, :], lhsT=act_src[:, o : o + w], rhs=w_sb[w_idx][:],
                            start=True, stop=True,
                        )
                        if t % 2:
                            nc.vector.tensor_copy(z_own[:w, t, :], zp[:w, :])
                        else:
                            nc.scalar.copy(z_own[:w, t, :], zp[:w, :])
                        nc.sync.dma_start(
                            out=z_shard[o : o + w, :], in_=z_own[:w, t, :]
                        )
                    t0 = piece_end_tile[p]
                    nc.gpsimd.collective_compute(
                        "AllGather",
                        mybir.AluOpType.bypass,
                        replica_groups=[list(range(N_CORES))],
                        ins=[z_shard[p * PIECE : (p + 1) * PIECE, :].opt()],
                        outs=[z_pieces[w_idx][p][:].opt()],
                    )

            qrr = [0]  # round-robin queue counter (persists across layers)

            def conv_layer(lyr):
                """gather + S-matmul segment sum into conv_sb; bias; stats."""
                for gi, ginfo in enumerate(groups):
                    lo = ginfo["lo"]
                    hi = ginfo["hi"]
                    gmap = {}
                    smap = {}
                    for nm, run in (("lo", lo), ("hi", hi)):
                        gpool = glo_p if nm == "lo" else ghi_p
                        spool = slo_p if nm == "lo" else shi_p
                        src_ap = z_pieces[lyr][0 if nm == "lo" else 1][:, :]
                        for (cs0, cns) in run["calls"]:
                            nblk = cns // TILE
                            gbuf = gpool.tile([128, MAXBLK, F], fp16, tag=f"g{nm}")
                            nc.gpsimd.dma_gather(
                                gbuf[:, :nblk, :],
                                src_ap,
                                idx_sb[:, cs0 // 16 : (cs0 + cns) // 16],
                                cns,
                                cns,
                                F,
                                queue_num=qrr[0] % NQ,
                            )
                            qrr[0] += 1
                            sbuf = spool.tile([128, 1024], fp16, tag=f"s{nm}")
                            nc.sync.dma_start(
                                out=sbuf[:, :cns], in_=s_in[:, cs0 : cs0 + cns]
                            )
                            for j in range(nblk):
                                gmap[cs0 // TILE + j] = (gbuf, j)
                                smap[cs0 // TILE + j] = (sbuf, j)
                    for ti, t in enumerate(ginfo["tiles"]):
                        # blocks for tile t: (bufs, local block idx, global block)
                        blocks = []
                        for nm, run in (("lo", lo), ("hi", hi)):
                            s0, nb = run["tile_blocks"][ti]
                            for j in range(nb):
                                gb = s0 // TILE + j
                                blocks.append(gmap[gb] + smap[gb] + (gb,))
                        cps = conv_ps.tile([F, TILE], fp32, tag="convps")
                        # self-loop diag block first (no gather dependency)
                        nc.tensor.matmul(
                            cps[:],
                            lhsT=z_own[:, t, :],
                            rhs=sdiag_sb[:, t * TILE : (t + 1) * TILE],
                            start=True,
                            stop=(len(blocks) == 0),
                        )
                        for bi, (gbuf, lb, sbuf, slb, gb) in enumerate(blocks):
                            nc.tensor.matmul(
                                cps[:],
                                lhsT=gbuf[:, lb, :],
                                rhs=sbuf[:, slb * TILE : (slb + 1) * TILE],
                                start=False,
                                stop=(bi == len(blocks) - 1),
                            )
                        tw = LAST_W if t == NTILE - 1 else TILE
                        o = t * TILE
                        nc.scalar.activation(
                            out=conv_sb[:, o : o + tw],
                            in_=cps[:, :tw],
                            func=AF.Identity,
                            bias=b_sb[lyr][:],
                            scale=1.0,
                        )
                        nc.vector.bn_stats(
                            stats6[:, t, :], conv_sb[:, o : o + tw]
                        )

            def bn_relu(lyr, act_out):
                """global BN stats allreduce + fused scale/shift/relu -> act_out fp16."""
                aggr = small.tile([F, 2], fp32, tag="aggr")
                nc.vector.bn_aggr(aggr[:], stats6[:])
                st = small.tile([F, 2], fp32, tag="stats")
                # sum = mean * SHARD; sumsq = (var + mean^2) * SHARD
                nc.vector.tensor_scalar_mul(st[:, 0:1], aggr[:, 0:1], float(SHARD))
                m2 = small.tile([F, 1], fp32, tag="m2")
                nc.vector.tensor_tensor(
                    m2[:], aggr[:, 0:1], aggr[:, 0:1], op=OP.mult
                )
                nc.vector.tensor_tensor(m2[:], aggr[:, 1:2], m2[:], op=OP.add)
                nc.vector.tensor_scalar_mul(st[:, 1:2], m2[:], float(SHARD))
                nc.sync.dma_start(out=stats_locs[lyr][:], in_=st[:])
                nc.gpsimd.collective_compute(
                    "AllReduce",
                    OP.add,
                    replica_groups=[list(range(N_CORES))],
                    ins=[stats_locs[lyr][:].opt()],
                    outs=[stats_globs[lyr][:].opt()],
                )
                stg = small.tile([F, 2], fp32, tag="statsg")
                nc.sync.dma_start(out=stg[:], in_=stats_globs[lyr][:])
                mean = small.tile([F, 1], fp32, tag="mean")
                nc.vector.tensor_scalar_mul(mean[:], stg[:, 0:1], 1.0 / N)
                ex2 = small.tile([F, 1], fp32, tag="ex2")
                nc.vector.tensor_scalar_mul(ex2[:], stg[:, 1:2], 1.0 / N)
                var = small.tile([F, 1], fp32, tag="var")
                nc.vector.tensor_tensor(var[:], mean[:], mean[:], op=OP.mult)
                nc.vector.tensor_sub(var[:], ex2[:], var[:])
                nc.vector.tensor_scalar_add(var[:], var[:], BN_EPS)
                std = small.tile([F, 1], fp32, tag="std")
                nc.scalar.sqrt(std[:], var[:])
                rstd = small.tile([F, 1], fp32, tag="rstd")
                nc.vector.reciprocal(rstd[:], std[:])
                scale = small.tile([F, 1], fp32, tag="scale")
                nc.vector.tensor_mul(scale[:], rstd[:], g_sb[lyr][:])
                shift = small.tile([F, 1], fp32, tag="shift")
                nc.vector.tensor_mul(shift[:], mean[:], scale[:])
                nc.vector.tensor_sub(shift[:], be_sb[lyr][:], shift[:])
                nc.scalar.activation(
                    out=act_out[:],
                    in_=conv_sb[:],
                    func=AF.Relu,
                    bias=shift[:],
                    scale=scale[:],
                )

            nc.vector.memset(actB[:], 0.0)
            nc.vector.memset(conv_sb[:], 0.0)
            # ---- layer 1 ----
            produce_z(actA, 0)
            conv_layer(0)
            bn_relu(0, actB)
            # ---- layer 2 ----
            produce_z(actB, 1)
            conv_layer(1)
            bn_relu(1, actA)
            # jk12 = max(act1, act2) -> actB
            nc.vector.tensor_max(actB[:], actB[:], actA[:])
            # ---- layer 3 ----
            produce_z(actA, 2)
            conv_layer(2)
            # conv3 -> fp16 into actA, jk = max(jk12, conv3) -> actB
            nc.scalar.copy(actA[:], conv_sb[:])
            nc.vector.tensor_max(actB[:], actB[:], actA[:])
            # ---- projection ----
            zchunks = []
            o = 0
            while o < SHARD:
                w = min(ZCHUNK, SHARD - o)
                zchunks.append((o, w))
                o += w
            for (o, w) in zchunks:
                pp = z_ps.tile([F, ZCHUNK], fp32, tag="zps")
                nc.tensor.matmul(
                    pp[:OUTF, :w], lhsT=wp_sb[:], rhs=actB[:, o : o + w],
                    start=True, stop=True,
                )
                po = rstage.tile([OUTF, ZCHUNK], fp32, tag="pout")
                nc.scalar.activation(
                    out=po[:, :w], in_=pp[:OUTF, :w], func=AF.Identity,
                    bias=bp_sb[:], scale=1.0,
                )
                nc.sync.dma_start(out=out_ext[:, o : o + w], in_=po[:, :w])

    nc.compile()
    return nc


_CACHE = {}
_last_in_maps = None


def kernel(**inputs):
    from concourse.bass_utils import run_bass_kernel_spmd

    x = np.asarray(inputs["x"], dtype=np.float32)
    edge_index = np.asarray(inputs["edge_index"])

    ck = hash(edge_index.tobytes())
    if ck not in _CACHE:
        structure, per_core = _preprocess(edge_index)
        nc = _build(structure)
        _CACHE[ck] = (structure, per_core, nc)
    structure, per_core, nc = _CACHE[ck]

    in_maps = []
    for c in range(N_CORES):
        xc = x[c * SHARD : (c + 1) * SHARD].astype(np.float16)
        m = {
            "xT": np.ascontiguousarray(xc.T),
            "idx": per_core[c]["idx"],
            "S": per_core[c]["S"],
            "Sdiag": per_core[c]["Sdiag"],
            "W1": np.asarray(inputs["W1"], np.float16),
            "W2": np.asarray(inputs["W2"], np.float16),
            "W3": np.asarray(inputs["W3"], np.float16),
            "Wp": np.asarray(inputs["Wp"], np.float16),
            "b1": np.asarray(inputs["b1"], np.float32).reshape(F, 1),
            "b2": np.asarray(inputs["b2"], np.float32).reshape(F, 1),
            "b3": np.asarray(inputs["b3"], np.float32).reshape(F, 1),
            "bp": np.asarray(inputs["bp"], np.float32).reshape(OUTF, 1),
            "g1": np.asarray(inputs["g1"], np.float32).reshape(F, 1),
            "g2": np.asarray(inputs["g2"], np.float32).reshape(F, 1),
            "be1": np.asarray(inputs["be1"], np.float32).reshape(F, 1),
            "be2": np.asarray(inputs["be2"], np.float32).reshape(F, 1),
        }
        in_maps.append(m)

    global _last_in_maps
    _last_in_maps = in_maps
    res = run_bass_kernel_spmd(nc, in_maps, core_ids=list(range(N_CORES)))
    out = np.empty((N, OUTF), dtype=np.float32)
    for c in range(N_CORES):
        out[c * SHARD : (c + 1) * SHARD] = res.results[c]["outT"].T
    return out
